# revision 26
# baseline (speedup 1.0000x reference)
"""Single-head attention with LoRA-folded projections on 8 TRN2 NeuronCores.

Problem: nn_Attention_Head (B=8, S=2048, EMB=1024, HEAD=64, RANK=8).
Sharding: data-parallel over batch — core b computes batch element b.

Math (per batch):
  Weff_x = Wx + 2.0 * (Bx @ Ax)            (LoRA folded on host — exact algebra)
  q = x @ Weff_q^T ; k = x @ Weff_k^T ; v = x @ Weff_v^T
  S = q @ k^T / 8, masked where tokMrk==0, softmax over keys, out = S @ v

Keys with tokMrk==0 contribute exactly zero to the masked softmax, so k/v are
only computed over the COMPACTED unmasked tokens (~1024 of 2048), gathered on
the host and padded to KC=1152.  Pad positions get the -480 mask bias (row 64
of kTb) -> exp == 0.

Device pipeline (per core):
  1. xkT (compacted, k-block-major) and xT (q-block-major) arrive
     pre-transposed in bf16; one contiguous DMA per block so the projections
     track the DMAs.
  2. Packed [Wk|Wv] projection (M=128) per k-block -> k rows 0-63 -> kTb,
     v rows 64-127 -> staged, PE-transposed into v_nat [tok, 64] with a ones
     column.  kTb row 64 = mask bias; qT1 row 64 = ones (S^T matmul adds the
     mask for free, K = 65; the PV ones column yields softmax denominators).
  3. q projection in column-tiled block pairs (two M=64 blocks side by side
     in the PE array), odd blocks realigned by SBUF->SBUF DMA.
  4. Attention: one flat pipeline over (q-block, k-pair); S^T two pairs ahead
     of PV; one ACT exp instruction covers two k-tiles; q-projections and
     epilogues spread between PV pairs as PE filler.
  5. outT PE-transposed (fp32) to [q,65]; out = outT[:, :64] / outT[:, 64].
"""

import numpy as np
from contextlib import ExitStack

import ml_dtypes
import concourse.bass as bass
import concourse.mybir as mybir
import concourse.tile as tile
from concourse import bacc, bass_utils

B, S, EMB, HEAD = 8, 2048, 1024, 64
LORA_SCALE = 2.0
MASK_BIAS = -480.0     # pre-softmax-scale; * 0.125 -> -60 added to the logits
N_CORES = 8
KC = 1152              # compacted+padded key count (actual ~1024, binom(2048,.5))
KTC = KC // 128        # 9 k-tiles
QB = S // 512          # 4 q-blocks
NCH = EMB // 128       # 8 emb chunks
KB = [(0, 512), (512, 512), (1024, 128)]   # k/v projection N-blocks over KC
# k-tile pairs per q-block: 4 pairs of 2 + 1 single
PAIRS = [(0, 2), (2, 2), (4, 2), (6, 2), (8, 1)]
NPAIR = len(PAIRS)

F32 = mybir.dt.float32
BF16 = mybir.dt.bfloat16
EXP = mybir.ActivationFunctionType.Exp

# test.py can override these to enable tracing
RUN_KWARGS = {}


def build_nc():
    nc = bacc.Bacc("TRN2", target_bir_lowering=False, debug=False)

    xt_d = nc.dram_tensor("xt", [QB, 128, NCH, 512], BF16, kind="ExternalInput").ap()
    xtk_d = nc.dram_tensor("xtk", [128, NCH * KC], BF16, kind="ExternalInput").ap()
    wt_d = nc.dram_tensor("wt", [128, NCH, 3 * HEAD], BF16, kind="ExternalInput").ap()
    maskrow_d = nc.dram_tensor("maskrow", [1, KC], BF16, kind="ExternalInput").ap()
    onesrow_d = nc.dram_tensor("onesrow", [1, S], BF16, kind="ExternalInput").ap()
    onescol_d = nc.dram_tensor("onescol", [128, KTC, 1], BF16, kind="ExternalInput").ap()
    ident_d = nc.dram_tensor("ident", [128, 128], BF16, kind="ExternalInput").ap()
    identf_d = nc.dram_tensor("identf", [128, 128], F32, kind="ExternalInput").ap()
    out_d = nc.dram_tensor("out", [S, HEAD], F32, kind="ExternalOutput").ap()

    # column offsets of each k-block inside xtk (block-major: NCH chunks of kw)
    kb_off = []
    off = 0
    for k0, kw in KB:
        kb_off.append(off)
        off += NCH * kw

    with tile.TileContext(nc) as tc, ExitStack() as ctx:
        consts = ctx.enter_context(tc.tile_pool(name="consts", bufs=1))
        xtp = ctx.enter_context(tc.tile_pool(name="xt", bufs=1))
        qkv = ctx.enter_context(tc.tile_pool(name="qkv", bufs=1))
        ptp = ctx.enter_context(tc.tile_pool(name="pt", bufs=6))
        osum = ctx.enter_context(tc.tile_pool(name="osum", bufs=2))
        oout = ctx.enter_context(tc.tile_pool(name="oout", bufs=4))

        # PSUM: sc 2x1 + st 2x2 + po 2x1 = 8 banks
        ps_sc = ctx.enter_context(tc.tile_pool(name="ps_sc", bufs=2, space="PSUM"))
        ps_st = ctx.enter_context(tc.tile_pool(name="ps_st", bufs=2, space="PSUM"))
        ps_o = ctx.enter_context(tc.tile_pool(name="ps_o", bufs=2, space="PSUM"))

        # small consts on the ACT HWDGE ring; x traffic on the SP ring in
        # exact need-order: xkT blocks -> xT q-blocks
        wt_sb = consts.tile([128, NCH, 3 * HEAD], BF16)
        nc.scalar.dma_start(out=wt_sb[:], in_=wt_d)
        ident = consts.tile([128, 128], BF16)
        nc.scalar.dma_start(out=ident[:], in_=ident_d)
        identf = consts.tile([128, 128], F32)
        nc.scalar.dma_start(out=identf[:], in_=identf_d)

        qT1 = qkv.tile([HEAD + 1, S], BF16)
        kTb = qkv.tile([HEAD + 1, KC], BF16)
        vT64 = qkv.tile([128, KC], BF16)     # v^T staged on partitions 64-127
        v1 = qkv.tile([128, KTC, HEAD + 1], BF16)
        nc.scalar.dma_start(out=qT1[HEAD:HEAD + 1, :], in_=onesrow_d)
        nc.scalar.dma_start(out=kTb[HEAD:HEAD + 1, :], in_=maskrow_d)
        nc.scalar.dma_start(out=v1[:, :, HEAD:HEAD + 1], in_=onescol_d)

        xtk_sb = xtp.tile([128, NCH * KC], BF16)
        xt_sb = xtp.tile([128, QB, NCH, 512], BF16)

        def dma_xtk(bi):
            o0, kw = kb_off[bi], KB[bi][1]
            nc.sync.dma_start(out=xtk_sb[:, o0:o0 + NCH * kw],
                              in_=xtk_d[:, o0:o0 + NCH * kw])

        def dma_xt(nb):
            nc.sync.dma_start(out=xt_sb[:, nb, :, :], in_=xt_d[nb])

        dma_xtk(0)
        dma_xt(0)
        dma_xtk(1)
        dma_xtk(2)
        dma_xt(1)
        dma_xt(2)
        dma_xt(3)

        def xtk_slice(bi, c):
            o0, kw = kb_off[bi], KB[bi][1]
            return xtk_sb[:, o0 + c * kw: o0 + (c + 1) * kw]

        # ---- k/v projection per k-block (tracks its DMA), then v_nat ----
        def kv_block(bi):
            k0, kw = KB[bi]
            pkv = ps_st.tile([128, 2, 512], F32, tag="st", name=f"pkv{bi}")
            for c in range(NCH):
                nc.tensor.matmul(
                    out=pkv[:, 0, 0:kw],
                    lhsT=wt_sb[:, c, HEAD:3 * HEAD],
                    rhs=xtk_slice(bi, c),
                    start=(c == 0), stop=(c == NCH - 1),
                )
            nc.vector.tensor_copy(kTb[0:HEAD, k0:k0 + kw], pkv[0:HEAD, 0, 0:kw])
            nc.vector.tensor_copy(vT64[HEAD:128, k0:k0 + kw], pkv[HEAD:128, 0, 0:kw])
            # transpose this block's v k-tiles into v_nat
            nkt = kw // 128
            pw = ps_sc.tile([128, 1024], BF16, tag="sc", name=f"pw{bi}")
            for j in range(nkt):
                kt = k0 // 128 + j
                nc.tensor.matmul(
                    out=pw[:, j * HEAD:(j + 1) * HEAD],
                    lhsT=vT64[HEAD:128, kt * 128:(kt + 1) * 128],
                    rhs=ident[HEAD:128, HEAD:128],
                    is_transpose=True,
                    start=(j == 0), stop=(j == nkt - 1),
                )
            vsrc = pw[:, 0:nkt * HEAD].rearrange("p (j f) -> p j f", j=nkt)
            nc.vector.tensor_copy(v1[:, k0 // 128:k0 // 128 + nkt, 0:HEAD], vsrc)

        # ---- q projection: column-tiled block pairs (even blocks on T0 ->
        # psum rows 0-63, odd blocks on T1 -> psum rows 64-127, own banks) ----
        def q_proj(nb):
            pq = ps_sc.tile([128, 512], F32, tag="sc", name=f"pq{nb}")
            for c in range(NCH):
                nc.tensor.matmul(
                    out=pq[0:HEAD, :],
                    lhsT=wt_sb[:, c, 0:HEAD],
                    rhs=xt_sb[:, nb, c, :],
                    start=(c == 0), stop=(c == NCH - 1),
                )
            nc.vector.tensor_copy(qT1[0:HEAD, nb * 512:(nb + 1) * 512], pq[0:HEAD, :])

        # ---- attention: flat pipeline over (q-block, k-pair) ----
        NPT = QB * NPAIR
        po_t = {}
        ptiles = {}

        def emit_pair(i):
            qb, kp = divmod(i, NPAIR)
            kt0, np_ = PAIRS[kp]
            pst = ps_st.tile([128, 2, 512], F32, tag="st", name=f"pst{i}")
            for j in range(np_):
                kt = kt0 + j
                nc.tensor.matmul(
                    out=pst[:, j, :],
                    lhsT=kTb[:, kt * 128:(kt + 1) * 128],
                    rhs=qT1[:, qb * 512:(qb + 1) * 512],
                    start=True, stop=True,
                )
            pt_t = ptp.tile([128, 2, 512], BF16, tag="pt", name=f"pt{i}")
            nc.scalar.activation(
                out=pt_t[:, 0:np_, :], in_=pst[:, 0:np_, :], func=EXP,
                scale=1.0 / np.sqrt(HEAD))
            ptiles[i] = pt_t

        def pv(i):
            qb, kp = divmod(i, NPAIR)
            kt0, np_ = PAIRS[kp]
            pt_t = ptiles.pop(i)
            for j in range(np_):
                kt = kt0 + j
                nc.tensor.matmul(
                    out=po_t[qb][:],
                    lhsT=v1[:, kt, :],
                    rhs=pt_t[:, j, :],
                    start=(kt == 0), stop=(kt == KTC - 1),
                )

        os_tiles = {}

        def epilogue_part(qb, half):
            if half == 0:
                os_sb = osum.tile([HEAD + 1, 512], F32, tag="os", name=f"os{qb}")
                os_tiles[qb] = os_sb
                nc.vector.tensor_copy(os_sb[:], po_t.pop(qb)[:])
            os_sb = os_tiles[qb]
            for j in ((0, 1) if half == 0 else (2, 3)):
                pt2 = ps_sc.tile([128, 512], F32, tag="sc", name=f"pt2_{qb}_{j}")
                nc.tensor.matmul(
                    out=pt2[:, 0:HEAD + 1],
                    lhsT=os_sb[:, j * 128:(j + 1) * 128],
                    rhs=identf[0:HEAD + 1, 0:HEAD + 1],
                    is_transpose=True,
                    start=True, stop=True,
                )
                inv = oout.tile([128, 1], F32, tag="inv", name=f"inv{qb}_{j}")
                nc.vector.reciprocal(inv[:], pt2[:, HEAD:HEAD + 1])
                ob = oout.tile([128, HEAD], F32, tag="ob", name=f"ob{qb}_{j}")
                nc.vector.tensor_scalar_mul(ob[:], pt2[:, 0:HEAD], inv[:])
                r0 = qb * 512 + j * 128
                nc.sync.dma_start(out=out_d[r0:r0 + 128, :], in_=ob[:])

        kv_block(0)
        q_proj(0)
        emit_pair(0)
        emit_pair(1)
        for i in range(NPT):
            qb, kp = divmod(i, NPAIR)
            if kp == 0:
                po_t[qb] = ps_o.tile([HEAD + 1, 512], F32, tag="po", name=f"po{qb}")
            pv(i)
            if qb == 0:
                if kp == 0:
                    kv_block(1)
                elif kp == 1:
                    kv_block(2)
                elif kp == 2:
                    q_proj(1)
            else:
                if kp == 0:
                    epilogue_part(qb - 1, 0)
                elif kp == 1:
                    epilogue_part(qb - 1, 1)
                elif kp == 2 and qb == 1:
                    q_proj(2)
                elif kp == 2 and qb == 2:
                    q_proj(3)
            if i + 2 < NPT:
                emit_pair(i + 2)
        epilogue_part(QB - 1, 0)
        epilogue_part(QB - 1, 1)

    nc.compile()
    return nc


def prep_inputs(batEmb, tokMrk, Wq, Wk, Wv, Aq, Bq, Ak, Bk, Av, Bv):
    """Fold LoRA into the base weights, compact keys, lay out per-core maps."""
    ws = []
    for W, A, Bm in ((Wq, Aq, Bq), (Wk, Ak, Bk), (Wv, Av, Bv)):
        ws.append(W.astype(np.float64) + LORA_SCALE * (Bm.astype(np.float64) @ A.astype(np.float64)))
    wcat = np.concatenate(ws, axis=0).astype(np.float32)          # [192, 1024]
    wt = np.ascontiguousarray(
        wcat.T.reshape(NCH, 128, 3 * HEAD).transpose(1, 0, 2))    # [128, NCH, 192]
    wt = wt.astype(ml_dtypes.bfloat16)
    ident = np.eye(128, dtype=ml_dtypes.bfloat16)
    identf = np.eye(128, dtype=np.float32)

    in_maps = []
    for b in range(B):
        xb = batEmb[b].astype(ml_dtypes.bfloat16)                 # [S, EMB]
        xt = np.ascontiguousarray(
            xb.T.reshape(NCH, 128, QB, 512).transpose(2, 1, 0, 3))  # [QB,128,NCH,512]
        idx = np.nonzero(tokMrk[b])[0]
        cnt = len(idx)
        assert cnt <= KC, f"batch {b}: {cnt} unmasked keys > KC={KC}"
        idx_pad = np.concatenate([idx, np.zeros(KC - cnt, np.int64)])
        xkT = xb[idx_pad, :].T                                    # [EMB, KC]
        # block-major: for each k-block, [128, NCH, kw] flattened
        blocks = []
        for k0, kw in KB:
            blk = xkT[:, k0:k0 + kw].reshape(NCH, 128, kw).transpose(1, 0, 2)
            blocks.append(blk.reshape(128, NCH * kw))
        xtk = np.ascontiguousarray(np.concatenate(blocks, axis=1))  # [128, NCH*KC]
        maskrow = np.where(np.arange(KC) < cnt, np.float32(0.0),
                           np.float32(MASK_BIAS)).reshape(1, KC)
        in_maps.append({
            "xt": xt,
            "xtk": xtk,
            "wt": wt,
            "maskrow": maskrow.astype(ml_dtypes.bfloat16),
            "onesrow": np.ones((1, S), ml_dtypes.bfloat16),
            "onescol": np.ones((128, KTC, 1), ml_dtypes.bfloat16),
            "ident": ident,
            "identf": identf,
        })
    return in_maps


_CACHED_NC = None


def kernel(**inputs):
    global _CACHED_NC
    if _CACHED_NC is None:
        _CACHED_NC = build_nc()
    nc = _CACHED_NC
    in_maps = prep_inputs(**{k: np.asarray(v) for k, v in inputs.items()})
    res = bass_utils.run_bass_kernel_spmd(
        nc, in_maps, core_ids=list(range(N_CORES)), **RUN_KWARGS)
    kernel.last_results = res
    return np.stack([res.results[b]["out"] for b in range(N_CORES)])
